# revision 12
# baseline (speedup 1.0000x reference)
"""CorrelationLayer (81-shift local correlation) on 8 Trainium2 NeuronCores.

Full inputs: feat1, feat2 [4, 128, 184, 320] fp32.
Full output: [4, 81, 184, 320] fp32,
  out[b, (dy+4)*9+(dx+4), y, x] = <f1n[b,:,y,x], f2n[b,:,y-dy,x-dx]>
  (features L2-normalized over C; f2 zero-padded outside the frame).

Sharding: 8 cores = batch(4) x W-halves(2).  Each core gets
  f1 shard [128, 184, 160] and f2 shard [128, 192, 168] (4-pixel
  zero-padded halo on all spatial sides baked in on the host), both
  pre-cast to bf16 on the host (the on-device pipeline is bf16 anyway,
  and it halves the input HBM traffic).

Per-core kernel:
  Phase 0 (normalize, in place, natural [C, rows, w] layout):
    sq = x*x                      (GPSIMD, idle otherwise)
    s  = colsum(sq) bcast to C    (one PE matmul vs an all-ones [C,128]
                                   stationary -> PSUM [C, chunk], no
                                   single-lane [1,N] intermediates)
    inv = Dsqrt(s*0.25 + eps)     (ACT; Dsqrt(u)=0.5*u^-1/2, so the
                                   0.25 scale makes it exactly s^-1/2)
    x *= inv                      (DVE, in place)
  Phase 1: for each 8x16-pixel block one PE matmul
    [C,128pix] x [C, 16x24 halo] -> PSUM [128, 384] all-pairs tile that
    contains every (pixel, shift) correlation exactly once; evacuate
    PSUM -> SBUF bf16 (alternating ACT/DVE) into a per-block-row buffer
    and store one [128, 10*384] DMA per block row (DRAM layout is
    partition-major so each partition writes 7.7 KB contiguous).

The host gathers windows from the sheared tiles into the [81, H, W]
layout during unshard (a fixed index permutation).  On-chip de-shear is
not performed because extraction needs per-partition column offsets,
which only partition-fractional DMA APs can express and those are both
broken >32 partitions and descriptor-bound; all FLOPs and the
normalization run on-device.
"""

from contextlib import ExitStack

import numpy as np
import ml_dtypes

import concourse.bass as bass
import concourse.bacc as bacc
import concourse.tile as tile
from concourse import mybir
from concourse.bass_utils import run_bass_kernel_spmd

F32 = mybir.dt.float32
BF16 = mybir.dt.bfloat16

# problem constants (hardcoded per harness contract)
B, C, H, W = 4, 128, 184, 320
ROWS, WIDTH = 184, 160          # per-core shard (W-half)
PY, PX = 8, 16                  # pixel block
HY, HX = PY + 8, PX + 8         # halo block (16 x 24)
NHALO = HY * HX                 # 384
NBY, NBX = ROWS // PY, WIDTH // PX
NBLK = NBY * NBX                # 230

_compiled = {}


def _build_kernel(nc, f1, f2, out):
    tc_ctx = tile.TileContext(nc)
    with tc_ctx as tc, ExitStack() as ctx:
        rows, width = ROWS, WIDTH
        w2, rows2 = width + 8, rows + 8
        ctx.enter_context(nc.allow_low_precision(
            reason="bf16 feature/inv-norm pipeline within correlation tolerance"))

        persist = ctx.enter_context(tc.tile_pool(name="persist", bufs=1))
        temps = ctx.enter_context(tc.tile_pool(name="temps", bufs=3))
        psum_n = ctx.enter_context(
            tc.tile_pool(name="psum_n", bufs=2, space="PSUM"))
        psum_m = ctx.enter_context(
            tc.tile_pool(name="psum_m", bufs=2, space="PSUM"))
        smpool = ctx.enter_context(tc.tile_pool(name="sm", bufs=2))

        # f1 arrives block-major from the host: [C, NBLK, 128] where the
        # last dim is (iy, ix) within an 8x16 block.  Normalization is
        # pointwise, so phase 0 works on flat contiguous chunks; phase 1
        # lhsT is a contiguous [C, 128] slice.  f2 stays in natural
        # layout (its rhs windows overlap block boundaries).
        f1b = persist.tile([C, NBLK * 128], BF16)
        f2n = persist.tile([C, rows2, w2], BF16)
        allones = persist.tile([C, C], BF16)
        nc.vector.memset(allones, 1.0)
        eps_t = persist.tile([C, 1], F32)
        nc.vector.memset(eps_t, 1e-12)

        # raw loads, interleaved chunks so phase 0 can start early
        NLD = 4
        n1 = NBLK * 128
        for i in range(NLD):
            c0 = (n1 * i // NLD) // 512 * 512
            c1 = n1 if i == NLD - 1 else (n1 * (i + 1) // NLD) // 512 * 512
            nc.sync.dma_start(out=f1b[:, c0:c1], in_=f1[:, c0:c1])
            r0 = (rows2 * i) // NLD
            r1 = (rows2 * (i + 1)) // NLD
            nc.sync.dma_start(out=f2n[:, r0:r1], in_=f2[:, r0:r1])

        def phase0_chunk(xf0, xf1, c0, c1, sq_gpsimd, mult_gpsimd, tag):
            # normalize two contiguous sub-chunks (<=512 each) in place;
            # paired into one 2-bank PSUM tile so rsqrt and (when shapes
            # match) the multiply run once per pair.
            sq = temps.tile([C, 2, 512], BF16, tag=f"sq{tag}")
            seng = nc.gpsimd if sq_gpsimd else nc.vector
            seng.tensor_mul(out=sq[:, 0, :c0], in0=xf0, in1=xf0)
            if xf1 is not None:
                seng.tensor_mul(out=sq[:, 1, :c1], in0=xf1, in1=xf1)
            pn = psum_n.tile([C, 2, 512], F32, tag="pn")
            nc.tensor.matmul(pn[:, 0, :c0], allones, sq[:, 0, :c0],
                             start=True, stop=True)
            if xf1 is not None:
                nc.tensor.matmul(pn[:, 1, :c1], allones, sq[:, 1, :c1],
                                 start=True, stop=True)
            inv = temps.tile([C, 2, 512], BF16, tag=f"inv{tag}")
            meng = nc.gpsimd if mult_gpsimd else nc.vector
            # |s + eps|^-1/2 == rsqrt(s + eps) for s >= 0
            if xf1 is not None and c0 == c1:
                nc.scalar.activation(
                    out=inv[:, :, :c0], in_=pn[:, :, :c0],
                    func=mybir.ActivationFunctionType.Abs_reciprocal_sqrt,
                    scale=1.0, bias=eps_t)
                meng.tensor_mul(out=xf0, in0=xf0, in1=inv[:, 0, :c0])
                meng.tensor_mul(out=xf1, in0=xf1, in1=inv[:, 1, :c1])
            else:
                for j, (xf, cc) in enumerate(((xf0, c0), (xf1, c1))):
                    if xf is None:
                        continue
                    nc.scalar.activation(
                        out=inv[:, j, :cc], in_=pn[:, j, :cc],
                        func=mybir.ActivationFunctionType.Abs_reciprocal_sqrt,
                        scale=1.0, bias=eps_t)
                    meng.tensor_mul(out=xf, in0=xf, in1=inv[:, j, :cc])

        # phase-0 pair generators, emitted band-interleaved with phase 1.
        # f2: 6-row pairs (2 x 504); squares on GpSimd, half the mults too
        f2_pairs = [(s, min(6, rows2 - s)) for s in range(0, rows2, 6)]
        # f1: 1024-col pairs, squares and mults on DVE
        f1_pairs = [(s, min(1024, n1 - s)) for s in range(0, n1, 1024)]
        state = {"f2": 0, "f1": 0}

        def ensure_f2(rows_needed):
            while state["f2"] * 6 < min(rows_needed, rows2):
                s, nr = f2_pairs[state["f2"]]
                h0 = min(3, nr)
                h1 = nr - h0
                xf0 = f2n[:, s:s + h0].rearrange("c r x -> c (r x)")
                xf1 = (f2n[:, s + h0:s + nr].rearrange("c r x -> c (r x)")
                       if h1 else None)
                phase0_chunk(xf0, xf1, h0 * w2, h1 * w2,
                             True, state["f2"] % 2 == 1, "b")
                state["f2"] += 1

        def ensure_f1(cols_needed):
            while state["f1"] * 1024 < min(cols_needed, n1):
                s, ncc = f1_pairs[state["f1"]]
                c0 = min(512, ncc)
                c1 = ncc - c0
                xf0 = f1b[:, s:s + c0]
                xf1 = f1b[:, s + c0:s + ncc] if c1 else None
                phase0_chunk(xf0, xf1, c0, c1, False, False, "a")
                state["f1"] += 1

        half = 0
        for by in range(NBY):
            ensure_f2(by * PY + HY)
            ensure_f1((by + 1) * NBX * 128)
            sm = smpool.tile([128, NBX * NHALO], BF16)
            for bx0 in range(0, NBX, 2):
                pm = psum_m.tile([128, 2, 512], F32)
                for j in range(2):
                    blk = by * NBX + bx0 + j
                    lhsT = f1b[:, blk * 128:(blk + 1) * 128]
                    rhs = f2n[:, by * PY:by * PY + HY,
                              (bx0 + j) * PX:(bx0 + j) * PX + HX]
                    nc.tensor.matmul(pm[:, j, :NHALO], lhsT, rhs,
                                     start=True, stop=True)
                dst = sm[:, bx0 * NHALO:(bx0 + 2) * NHALO]
                dst = dst.rearrange("p (j n) -> p j n", j=2)
                if half == 0:
                    nc.scalar.copy(out=dst, in_=pm[:, :, :NHALO])
                else:
                    nc.vector.tensor_copy(out=dst, in_=pm[:, :, :NHALO])
                half ^= 1
            nc.sync.dma_start(
                out=out[:, by * NBX * NHALO:(by + 1) * NBX * NHALO], in_=sm)


def _get_program():
    if "nc" not in _compiled:
        nc = bacc.Bacc("TRN2", target_bir_lowering=False, debug=False)
        f1 = nc.dram_tensor("f1", [C, NBLK * 128], BF16,
                            kind="ExternalInput").ap()
        f2 = nc.dram_tensor("f2", [C, ROWS + 8, WIDTH + 8], BF16,
                            kind="ExternalInput").ap()
        out = nc.dram_tensor("tiles", [128, NBLK * NHALO], BF16,
                             kind="ExternalOutput").ap()
        _build_kernel(nc, f1, f2, out)
        nc.compile()
        _compiled["nc"] = nc
    return _compiled["nc"]


def _host_extract(tiles):
    """Sheared tiles [NBLK, 128, 384] -> [81, ROWS, WIDTH] (fp32)."""
    v = tiles.reshape(NBY, NBX, PY, PX, HY, HX)
    out = np.empty((81, ROWS, WIDTH), np.float32)
    iy = np.arange(PY)[:, None]
    ix = np.arange(PX)[None, :]
    for dy in range(-4, 5):
        a = 4 - dy
        for dx in range(-4, 5):
            b = 4 - dx
            k = (dy + 4) * 9 + (dx + 4)
            g = v[:, :, iy, ix, iy + a, ix + b]      # [NBY, NBX, PY, PX]
            out[k] = g.transpose(0, 2, 1, 3).reshape(ROWS, WIDTH)
    return out


def run_cores(in_maps, **kwargs):
    """Compile once and run the SPMD kernel on cores 0-7."""
    nc = _get_program()
    return run_bass_kernel_spmd(nc, in_maps, core_ids=list(range(8)), **kwargs)


def make_in_maps(feat1, feat2):
    feat1 = np.asarray(feat1, dtype=np.float32).astype(ml_dtypes.bfloat16)
    feat2 = np.asarray(feat2, dtype=np.float32).astype(ml_dtypes.bfloat16)
    in_maps = []
    for b in range(B):
        f2p = np.zeros((C, H + 8, W + 8), ml_dtypes.bfloat16)
        f2p[:, 4:-4, 4:-4] = feat2[b]
        for h in range(2):
            x0 = WIDTH * h
            # f1 block-major: [C, NBY, PY, NBX, PX] -> [C, NBY, NBX, PY, PX]
            f1s = feat1[b, :, :, x0:x0 + WIDTH].reshape(C, NBY, PY, NBX, PX)
            f1s = f1s.transpose(0, 1, 3, 2, 4).reshape(C, NBLK * 128)
            in_maps.append({
                "f1": np.ascontiguousarray(f1s),
                "f2": np.ascontiguousarray(f2p[:, :, x0:x0 + WIDTH + 8]),
            })
    return in_maps


def assemble(results):
    out = np.empty((B, 81, H, W), np.float32)
    for i, res in enumerate(results):
        flat = np.asarray(list(res.values())[0]).astype(np.float32)
        # DRAM layout [128, NBLK*384] partition-major -> [NBLK, 128, 384]
        tiles = flat.reshape(128, NBLK, NHALO).transpose(1, 0, 2)
        b, h = i // 2, i % 2
        out[b, :, :, WIDTH * h:WIDTH * (h + 1)] = _host_extract(tiles)
    return out


def kernel(feat1, feat2):
    in_maps = make_in_maps(feat1, feat2)
    res = run_cores(in_maps)
    return assemble(res.results)


# revision 13
# speedup vs baseline: 1.3502x; 1.3502x over previous
"""CorrelationLayer (81-shift local correlation) on 8 Trainium2 NeuronCores.

Full inputs: feat1, feat2 [4, 128, 184, 320] fp32.
Full output: [4, 81, 184, 320] fp32,
  out[b, (dy+4)*9+(dx+4), y, x] = <f1n[b,:,y,x], f2n[b,:,y-dy,x-dx]>
  (features L2-normalized over C; f2 zero-padded outside the frame).

Sharding: 8 cores = batch(4) x W-halves(2).  Each core gets
  f1 shard [128, 184, 160] and f2 shard [128, 192, 168] (4-pixel
  zero-padded halo on all spatial sides baked in on the host), both
  pre-cast to bf16 on the host (the on-device pipeline is bf16 anyway,
  and it halves the input HBM traffic).

Per-core kernel:
  Phase 0 (normalize, in place, natural [C, rows, w] layout):
    sq = x*x                      (GPSIMD, idle otherwise)
    s  = colsum(sq) bcast to C    (one PE matmul vs an all-ones [C,128]
                                   stationary -> PSUM [C, chunk], no
                                   single-lane [1,N] intermediates)
    inv = Dsqrt(s*0.25 + eps)     (ACT; Dsqrt(u)=0.5*u^-1/2, so the
                                   0.25 scale makes it exactly s^-1/2)
    x *= inv                      (DVE, in place)
  Phase 1: for each 8x16-pixel block one PE matmul
    [C,128pix] x [C, 16x24 halo] -> PSUM [128, 384] all-pairs tile that
    contains every (pixel, shift) correlation exactly once; evacuate
    PSUM -> SBUF bf16 (alternating ACT/DVE) into a per-block-row buffer
    and store one [128, 10*384] DMA per block row (DRAM layout is
    partition-major so each partition writes 7.7 KB contiguous).

The host gathers windows from the sheared tiles into the [81, H, W]
layout during unshard (a fixed index permutation).  On-chip de-shear is
not performed because extraction needs per-partition column offsets,
which only partition-fractional DMA APs can express and those are both
broken >32 partitions and descriptor-bound; all FLOPs and the
normalization run on-device.
"""

from contextlib import ExitStack

import numpy as np
import ml_dtypes

import concourse.bass as bass
import concourse.bacc as bacc
import concourse.tile as tile
from concourse import mybir
from concourse.bass_utils import run_bass_kernel_spmd

F32 = mybir.dt.float32
BF16 = mybir.dt.bfloat16

# problem constants (hardcoded per harness contract)
B, C, H, W = 4, 128, 184, 320
ROWS, WIDTH = 184, 160          # per-core shard (W-half)
PY, PX = 8, 16                  # pixel block
HY, HX = PY + 8, PX + 8         # halo block (16 x 24)
NHALO = HY * HX                 # 384
NBY, NBX = ROWS // PY, WIDTH // PX
NBLK = NBY * NBX                # 230

_compiled = {}


def _build_kernel(nc, f1, f2, out):
    tc_ctx = tile.TileContext(nc)
    with tc_ctx as tc, ExitStack() as ctx:
        rows, width = ROWS, WIDTH
        w2, rows2 = width + 8, rows + 8
        ctx.enter_context(nc.allow_low_precision(
            reason="bf16 feature/inv-norm pipeline within correlation tolerance"))

        persist = ctx.enter_context(tc.tile_pool(name="persist", bufs=1))
        temps = ctx.enter_context(tc.tile_pool(name="temps", bufs=4))
        psum_n = ctx.enter_context(
            tc.tile_pool(name="psum_n", bufs=2, space="PSUM"))
        psum_m = ctx.enter_context(
            tc.tile_pool(name="psum_m", bufs=2, space="PSUM"))
        smpool = ctx.enter_context(tc.tile_pool(name="sm", bufs=3))

        # f1 arrives block-major from the host: [C, NBLK, 128] where the
        # last dim is (iy, ix) within an 8x16 block.  Normalization is
        # pointwise, so phase 0 works on flat contiguous chunks; phase 1
        # lhsT is a contiguous [C, 128] slice.  f2 stays in natural
        # layout (its rhs windows overlap block boundaries).
        f1b = persist.tile([C, NBLK * 128], BF16)
        f2n = persist.tile([C, rows2, w2], BF16)
        allones = persist.tile([C, C], BF16)
        nc.vector.memset(allones, 1.0)
        eps_t = persist.tile([C, 1], F32)
        nc.vector.memset(eps_t, 1e-12)

        # raw loads, interleaved chunks so phase 0 can start early
        NLD = 4
        n1 = NBLK * 128
        for i in range(NLD):
            c0 = (n1 * i // NLD) // 512 * 512
            c1 = n1 if i == NLD - 1 else (n1 * (i + 1) // NLD) // 512 * 512
            nc.sync.dma_start(out=f1b[:, c0:c1], in_=f1[:, c0:c1])
            r0 = (rows2 * i) // NLD
            r1 = (rows2 * (i + 1)) // NLD
            nc.sync.dma_start(out=f2n[:, r0:r1], in_=f2[:, r0:r1])

        def phase0_chunk(xf0, xf1, c0, c1, sq_gpsimd, mult_gpsimd, tag):
            # normalize two contiguous sub-chunks (<=512 each) in place;
            # paired into one 2-bank PSUM tile so rsqrt and (when shapes
            # match) the multiply run once per pair.
            sq = temps.tile([C, 2, 512], BF16, tag=f"sq{tag}")
            seng = nc.gpsimd if sq_gpsimd else nc.vector
            seng.tensor_mul(out=sq[:, 0, :c0], in0=xf0, in1=xf0)
            if xf1 is not None:
                seng.tensor_mul(out=sq[:, 1, :c1], in0=xf1, in1=xf1)
            pn = psum_n.tile([C, 2, 512], F32, tag="pn")
            nc.tensor.matmul(pn[:, 0, :c0], allones, sq[:, 0, :c0],
                             start=True, stop=True)
            if xf1 is not None:
                nc.tensor.matmul(pn[:, 1, :c1], allones, sq[:, 1, :c1],
                                 start=True, stop=True)
            inv = temps.tile([C, 2, 512], BF16, tag=f"inv{tag}")
            meng = nc.gpsimd if mult_gpsimd else nc.vector
            # |s + eps|^-1/2 == rsqrt(s + eps) for s >= 0
            if xf1 is not None and c0 == c1:
                nc.scalar.activation(
                    out=inv[:, :, :c0], in_=pn[:, :, :c0],
                    func=mybir.ActivationFunctionType.Abs_reciprocal_sqrt,
                    scale=1.0, bias=eps_t)
                meng.tensor_mul(out=xf0, in0=xf0, in1=inv[:, 0, :c0])
                meng.tensor_mul(out=xf1, in0=xf1, in1=inv[:, 1, :c1])
            else:
                for j, (xf, cc) in enumerate(((xf0, c0), (xf1, c1))):
                    if xf is None:
                        continue
                    nc.scalar.activation(
                        out=inv[:, j, :cc], in_=pn[:, j, :cc],
                        func=mybir.ActivationFunctionType.Abs_reciprocal_sqrt,
                        scale=1.0, bias=eps_t)
                    meng.tensor_mul(out=xf, in0=xf, in1=inv[:, j, :cc])

        # phase-0 pair generators, emitted band-interleaved with phase 1.
        # f2: 6-row pairs (2 x 504); squares on GpSimd, half the mults too
        f2_pairs = [(s, min(6, rows2 - s)) for s in range(0, rows2, 6)]
        # f1: 1024-col pairs, squares and mults on DVE
        f1_pairs = [(s, min(1024, n1 - s)) for s in range(0, n1, 1024)]
        state = {"f2": 0, "f1": 0}

        def ensure_f2(rows_needed):
            while state["f2"] * 6 < min(rows_needed, rows2):
                s, nr = f2_pairs[state["f2"]]
                h0 = min(3, nr)
                h1 = nr - h0
                xf0 = f2n[:, s:s + h0].rearrange("c r x -> c (r x)")
                xf1 = (f2n[:, s + h0:s + nr].rearrange("c r x -> c (r x)")
                       if h1 else None)
                phase0_chunk(xf0, xf1, h0 * w2, h1 * w2,
                             False, False, "b")
                state["f2"] += 1

        def ensure_f1(cols_needed):
            while state["f1"] * 1024 < min(cols_needed, n1):
                s, ncc = f1_pairs[state["f1"]]
                c0 = min(512, ncc)
                c1 = ncc - c0
                xf0 = f1b[:, s:s + c0]
                xf1 = f1b[:, s + c0:s + ncc] if c1 else None
                phase0_chunk(xf0, xf1, c0, c1, False, False, "a")
                state["f1"] += 1

        half = 0
        for by in range(NBY):
            ensure_f2(by * PY + HY)
            ensure_f1((by + 1) * NBX * 128)
            sm = smpool.tile([128, NBX * NHALO], BF16)
            for bx0 in range(0, NBX, 2):
                pm = psum_m.tile([128, 2, 512], F32)
                for j in range(2):
                    blk = by * NBX + bx0 + j
                    lhsT = f1b[:, blk * 128:(blk + 1) * 128]
                    rhs = f2n[:, by * PY:by * PY + HY,
                              (bx0 + j) * PX:(bx0 + j) * PX + HX]
                    nc.tensor.matmul(pm[:, j, :NHALO], lhsT, rhs,
                                     start=True, stop=True)
                dst = sm[:, bx0 * NHALO:(bx0 + 2) * NHALO]
                dst = dst.rearrange("p (j n) -> p j n", j=2)
                if half == 0:
                    nc.scalar.copy(out=dst, in_=pm[:, :, :NHALO])
                else:
                    nc.vector.tensor_copy(out=dst, in_=pm[:, :, :NHALO])
                half ^= 1
            nc.sync.dma_start(
                out=out[:, by * NBX * NHALO:(by + 1) * NBX * NHALO], in_=sm)


def _get_program():
    if "nc" not in _compiled:
        nc = bacc.Bacc("TRN2", target_bir_lowering=False, debug=False)
        f1 = nc.dram_tensor("f1", [C, NBLK * 128], BF16,
                            kind="ExternalInput").ap()
        f2 = nc.dram_tensor("f2", [C, ROWS + 8, WIDTH + 8], BF16,
                            kind="ExternalInput").ap()
        out = nc.dram_tensor("tiles", [128, NBLK * NHALO], BF16,
                             kind="ExternalOutput").ap()
        _build_kernel(nc, f1, f2, out)
        nc.compile()
        _compiled["nc"] = nc
    return _compiled["nc"]


def _host_extract(tiles):
    """Sheared tiles [NBLK, 128, 384] -> [81, ROWS, WIDTH] (fp32)."""
    v = tiles.reshape(NBY, NBX, PY, PX, HY, HX)
    out = np.empty((81, ROWS, WIDTH), np.float32)
    iy = np.arange(PY)[:, None]
    ix = np.arange(PX)[None, :]
    for dy in range(-4, 5):
        a = 4 - dy
        for dx in range(-4, 5):
            b = 4 - dx
            k = (dy + 4) * 9 + (dx + 4)
            g = v[:, :, iy, ix, iy + a, ix + b]      # [NBY, NBX, PY, PX]
            out[k] = g.transpose(0, 2, 1, 3).reshape(ROWS, WIDTH)
    return out


def run_cores(in_maps, **kwargs):
    """Compile once and run the SPMD kernel on cores 0-7."""
    nc = _get_program()
    return run_bass_kernel_spmd(nc, in_maps, core_ids=list(range(8)), **kwargs)


def make_in_maps(feat1, feat2):
    feat1 = np.asarray(feat1, dtype=np.float32).astype(ml_dtypes.bfloat16)
    feat2 = np.asarray(feat2, dtype=np.float32).astype(ml_dtypes.bfloat16)
    in_maps = []
    for b in range(B):
        f2p = np.zeros((C, H + 8, W + 8), ml_dtypes.bfloat16)
        f2p[:, 4:-4, 4:-4] = feat2[b]
        for h in range(2):
            x0 = WIDTH * h
            # f1 block-major: [C, NBY, PY, NBX, PX] -> [C, NBY, NBX, PY, PX]
            f1s = feat1[b, :, :, x0:x0 + WIDTH].reshape(C, NBY, PY, NBX, PX)
            f1s = f1s.transpose(0, 1, 3, 2, 4).reshape(C, NBLK * 128)
            in_maps.append({
                "f1": np.ascontiguousarray(f1s),
                "f2": np.ascontiguousarray(f2p[:, :, x0:x0 + WIDTH + 8]),
            })
    return in_maps


def assemble(results):
    out = np.empty((B, 81, H, W), np.float32)
    for i, res in enumerate(results):
        flat = np.asarray(list(res.values())[0]).astype(np.float32)
        # DRAM layout [128, NBLK*384] partition-major -> [NBLK, 128, 384]
        tiles = flat.reshape(128, NBLK, NHALO).transpose(1, 0, 2)
        b, h = i // 2, i % 2
        out[b, :, :, WIDTH * h:WIDTH * (h + 1)] = _host_extract(tiles)
    return out


def kernel(feat1, feat2):
    in_maps = make_in_maps(feat1, feat2)
    res = run_cores(in_maps)
    return assemble(res.results)
